# revision 3
# baseline (speedup 1.0000x reference)
"""Binary (N-ary) TreeLSTM layer on 8 Trainium2 NeuronCores.

Per level l (24 levels, strictly sequential through the h/c recurrence):
    gates = x_l @ W' + bias + [h_c0|h_c1] @ U'      (node-major, PSUM-accumulated)
    f1,f2,i,o = sigmoid(...), u = tanh(...)
    c_l = i*u + f1*c_c0 + f2*c_c1 ; h_l = o * tanh(c_l)

Sharding: node dim N=8192 split over 8 cores (1024 nodes each). After each
level, an AllGather exchanges every core's [1024, 256] h|c rows (plus a
per-rank init row for index -1) into a replicated [8200, 256] DRAM table;
the next level gathers its children's rows from that table with one
indirect DMA (indices remapped on host, baked per-core).

x is pre-transposed + bf16-cast on the host so it can feed matmul lhsT
directly; weights are pre-transposed/reordered and duplicated (f column
block twice) so W and U contributions accumulate into one PSUM layout:
cols [f1 f2 i o] (bank0, 512) | [u] (bank1, 128).
"""

import numpy as np
import ml_dtypes

L, N, DIN, DOUT = 24, 8192, 256, 128
NCORES = 8
NS = N // NCORES            # nodes per core
P = 128                     # partitions
BF16 = ml_dtypes.bfloat16

_CACHE = {}


def _build(levels, ns, n_cores):
    """Build + compile the per-core SPMD NEFF. Returns the bacc module."""
    import concourse.bass as bass
    import concourse.bacc as bacc
    import concourse.tile as tile
    import concourse.mybir as mybir
    from concourse.masks import make_identity

    f32 = mybir.dt.float32
    bf16 = mybir.dt.bfloat16
    i32 = mybir.dt.int32
    SIG = mybir.ActivationFunctionType.Sigmoid
    TANH = mybir.ActivationFunctionType.Tanh

    T = ns // P                       # node tiles per core (8)
    V = n_cores * (ns + 1)            # gather-table rows (8200)
    G = 640                           # gate columns: [f1 f2 i o | u]

    nc = bacc.Bacc("TRN2", target_bir_lowering=False, debug=False,
                   num_devices=n_cores)

    xT_in = nc.dram_tensor("xT", [levels, DIN, ns], bf16, kind="ExternalInput")
    gidx_in = nc.dram_tensor("gidx", [P, levels * 2 * T], i32, kind="ExternalInput")
    Wp_in = nc.dram_tensor("Wp", [DIN, G], bf16, kind="ExternalInput")
    Ut_in = nc.dram_tensor("Ut", [2 * DOUT, G], bf16, kind="ExternalInput")
    bias_in = nc.dram_tensor("bias", [1, G], f32, kind="ExternalInput")
    bias0_in = nc.dram_tensor("bias0", [1, G], f32, kind="ExternalInput")
    cinit_in = nc.dram_tensor("cinit", [1, DOUT], f32, kind="ExternalInput")
    initrow_in = nc.dram_tensor("initrow", [1, 2 * DOUT], f32, kind="ExternalInput")
    resh_out = nc.dram_tensor("resh", [levels, ns, DOUT], f32, kind="ExternalOutput")
    resc_out = nc.dram_tensor("resc", [levels, ns, DOUT], f32, kind="ExternalOutput")

    with tile.TileContext(nc) as tc:
        with (
            tc.tile_pool(name="const", bufs=1) as cp,
            tc.tile_pool(name="xp", bufs=3) as xp,
            tc.tile_pool(name="gp", bufs=2) as gp,
            tc.tile_pool(name="hp", bufs=3) as hp,
            tc.tile_pool(name="sp", bufs=2) as sp,
            tc.tile_pool(name="psum", bufs=2, space="PSUM") as psp,
            tc.tile_pool(name="dram", bufs=2, space="DRAM") as dp,
        ):
            # --- constants ---
            Wp0 = cp.tile([P, G], bf16)
            Wp1 = cp.tile([P, G], bf16)
            Ut0 = cp.tile([P, G], bf16)
            Ut1 = cp.tile([P, G], bf16)
            nc.sync.dma_start(out=Wp0[:], in_=Wp_in[0:P, :])
            nc.sync.dma_start(out=Wp1[:], in_=Wp_in[P:2 * P, :])
            nc.sync.dma_start(out=Ut0[:], in_=Ut_in[0:P, :])
            nc.sync.dma_start(out=Ut1[:], in_=Ut_in[P:2 * P, :])
            bias_t = cp.tile([1, G], f32)
            bias0_t = cp.tile([1, G], f32)
            cinit_t = cp.tile([1, DOUT], f32)
            nc.sync.dma_start(out=bias_t[:], in_=bias_in[:])
            nc.sync.dma_start(out=bias0_t[:], in_=bias0_in[:])
            nc.sync.dma_start(out=cinit_t[:], in_=cinit_in[:])
            gidx_t = cp.tile([P, levels * 2 * T], i32)
            nc.sync.dma_start(out=gidx_t[:], in_=gidx_in[:])
            ones1 = cp.tile([1, P], f32)
            nc.vector.memset(ones1[:], 1.0)
            ident = cp.tile([P, P], f32)
            make_identity(nc, ident[:])
            # broadcast c_init across partitions: ones^T @ c_init
            cin_ps = psp.tile([P, DOUT], f32, space="PSUM")
            nc.tensor.matmul(out=cin_ps[:], lhsT=ones1[:], rhs=cinit_t[:],
                             start=True, stop=True)
            cinit_b = cp.tile([P, DOUT], f32)
            nc.vector.tensor_copy(out=cinit_b[:], in_=cin_ps[:])

            # persistent collective bounce input: row 0 = init state
            cc_in = dp.tile([ns + 1, 2 * DOUT], f32, bufs=1)
            nc.sync.dma_start(out=cc_in[0:1, :], in_=initrow_in[:])

            prev_tbl = None
            for l in range(levels):
                xk0 = xp.tile([P, ns], bf16)
                xk1 = xp.tile([P, ns], bf16)
                nc.sync.dma_start(out=xk0[:], in_=xT_in[l, 0:P, :])
                nc.sync.dma_start(out=xk1[:], in_=xT_in[l, P:2 * P, :])

                if l > 0:
                    gath = gp.tile([P, T * 2 * 256], f32)
                    for j in range(2 * T):
                        jc = l * 2 * T + j
                        nc.gpsimd.indirect_dma_start(
                            out=gath[:, j * 256:(j + 1) * 256], out_offset=None,
                            in_=prev_tbl[:],
                            in_offset=bass.IndirectOffsetOnAxis(
                                ap=gidx_t[:, jc:jc + 1], axis=0),
                        )
                    gath4 = gath.rearrange("p (t c d) -> p t c d", t=T, c=2, d=256)

                sig_s = sp.tile([P, T * 512], f32)
                u_s = sp.tile([P, T * 128], f32)
                bt = bias0_t if l == 0 else bias_t

                for t in range(T):
                    ts = slice(t * P, (t + 1) * P)
                    if l > 0:
                        trp = psp.tile([P, 256], f32, space="PSUM")
                        nc.tensor.transpose(out=trp[:, 0:128],
                                            in_=gath4[:, t, 0, 0:128],
                                            identity=ident[:])
                        nc.tensor.transpose(out=trp[:, 128:256],
                                            in_=gath4[:, t, 1, 0:128],
                                            identity=ident[:])
                        hT = hp.tile([P, 256], bf16)
                        nc.vector.tensor_copy(out=hT[:], in_=trp[:])

                    pg0 = psp.tile([P, 512], f32, space="PSUM")
                    pg1 = psp.tile([P, 128], f32, space="PSUM")
                    last0 = l == 0
                    nc.tensor.matmul(out=pg0[:], lhsT=ones1[:], rhs=bt[:, 0:512],
                                     start=True, stop=False)
                    nc.tensor.matmul(out=pg0[:], lhsT=xk0[:, ts], rhs=Wp0[:, 0:512],
                                     start=False, stop=False)
                    nc.tensor.matmul(out=pg0[:], lhsT=xk1[:, ts], rhs=Wp1[:, 0:512],
                                     start=False, stop=last0)
                    nc.tensor.matmul(out=pg1[:], lhsT=ones1[:], rhs=bt[:, 512:640],
                                     start=True, stop=False)
                    nc.tensor.matmul(out=pg1[:], lhsT=xk0[:, ts], rhs=Wp0[:, 512:640],
                                     start=False, stop=False)
                    nc.tensor.matmul(out=pg1[:], lhsT=xk1[:, ts], rhs=Wp1[:, 512:640],
                                     start=False, stop=last0)
                    if l > 0:
                        nc.tensor.matmul(out=pg0[:], lhsT=hT[:, 0:128],
                                         rhs=Ut0[:, 0:512], start=False, stop=False)
                        nc.tensor.matmul(out=pg0[:], lhsT=hT[:, 128:256],
                                         rhs=Ut1[:, 0:512], start=False, stop=True)
                        nc.tensor.matmul(out=pg1[:], lhsT=hT[:, 0:128],
                                         rhs=Ut0[:, 512:640], start=False, stop=False)
                        nc.tensor.matmul(out=pg1[:], lhsT=hT[:, 128:256],
                                         rhs=Ut1[:, 512:640], start=False, stop=True)

                    nc.scalar.activation(out=sig_s[:, t * 512:(t + 1) * 512],
                                         in_=pg0[:], func=SIG)
                    nc.scalar.activation(out=u_s[:, t * 128:(t + 1) * 128],
                                         in_=pg1[:], func=TANH)

                # --- batched elementwise over the whole level ---
                sig4 = sig_s.rearrange("p (t g d) -> p t g d", t=T, g=4, d=128)
                f1v, f2v = sig4[:, :, 0, :], sig4[:, :, 1, :]
                iv, ov = sig4[:, :, 2, :], sig4[:, :, 3, :]
                uv = u_s.rearrange("p (t d) -> p t d", d=128)[:]
                if l > 0:
                    c0v = gath4[:, :, 0, 128:256]
                    c1v = gath4[:, :, 1, 128:256]
                else:
                    c0v = cinit_b[:].unsqueeze(1).to_broadcast([P, T, 128])
                    c1v = c0v

                hc = sp.tile([P, T * 256], f32)
                hc4 = hc.rearrange("p (t d) -> p t d", d=256)
                tiu = sp.tile([P, T * 128], f32)
                t2 = sp.tile([P, T * 128], f32)
                t3 = sp.tile([P, T * 128], f32)
                tnh = sp.tile([P, T * 128], f32)
                tiu3 = tiu.rearrange("p (t d) -> p t d", d=128)
                t23 = t2.rearrange("p (t d) -> p t d", d=128)
                t33 = t3.rearrange("p (t d) -> p t d", d=128)
                tnh3 = tnh.rearrange("p (t d) -> p t d", d=128)
                nc.vector.tensor_mul(out=tiu3[:], in0=iv, in1=uv)
                nc.vector.tensor_mul(out=t23[:], in0=f1v, in1=c0v)
                nc.vector.tensor_mul(out=t33[:], in0=f2v, in1=c1v)
                nc.vector.tensor_add(out=t23[:], in0=t23[:], in1=t33[:])
                nc.vector.tensor_add(out=hc4[:, :, 128:256], in0=tiu3[:], in1=t23[:])
                nc.scalar.activation(out=tnh3[:], in_=hc4[:, :, 128:256], func=TANH)
                nc.vector.tensor_mul(out=hc4[:, :, 0:128], in0=ov, in1=tnh3[:])

                # --- outputs + exchange ---
                nc.sync.dma_start(
                    out=resh_out[l].rearrange("(t p) d -> p t d", p=P),
                    in_=hc4[:, :, 0:128])
                nc.sync.dma_start(
                    out=resc_out[l].rearrange("(t p) d -> p t d", p=P),
                    in_=hc4[:, :, 128:256])
                if l < levels - 1:
                    nc.sync.dma_start(
                        out=cc_in[1:1 + ns, :].rearrange("(t p) d -> p t d", p=P),
                        in_=hc4[:])
                    tbl = dp.tile([V, 2 * DOUT], f32,
                                  addr_space="Shared" if n_cores > 4 else "Local")
                    nc.gpsimd.collective_compute(
                        "AllGather", mybir.AluOpType.bypass,
                        replica_groups=[list(range(n_cores))],
                        ins=[cc_in[:].opt()], outs=[tbl[:].opt()],
                    )
                    prev_tbl = tbl

    nc.compile()
    return nc


def _prep_shared(inputs):
    """Host-side weight reordering. Gate col order: [f1 f2 i o | u]."""
    W_w, W_b = inputs["W_w"], inputs["W_b"]
    U_f1, U_f2, U_iuo = inputs["U_f1"], inputs["U_f2"], inputs["U_iuo"]
    h_init, c_init = inputs["h_init"], inputs["c_init"]
    D = DOUT
    Wt = np.asarray(W_w).T  # [DIN, 4D] cols: f i u o
    Wf, Wi, Wu, Wo = Wt[:, 0:D], Wt[:, D:2 * D], Wt[:, 2 * D:3 * D], Wt[:, 3 * D:4 * D]
    Wp = np.concatenate([Wf, Wf, Wi, Wo, Wu], axis=1)          # [DIN, 640]
    b = np.asarray(W_b)
    bp = np.concatenate([b[0:D], b[0:D], b[D:2 * D], b[3 * D:4 * D],
                         b[2 * D:3 * D]])[None, :]             # [1, 640]
    Ut = np.concatenate([np.asarray(U_f1).T, np.asarray(U_f2).T,
                         np.asarray(U_iuo).T[:, 0:D],
                         np.asarray(U_iuo).T[:, 2 * D:3 * D],
                         np.asarray(U_iuo).T[:, D:2 * D]], axis=1)  # [2D, 640]
    hc0 = np.concatenate([np.asarray(h_init), np.asarray(h_init)], axis=1)  # [1, 2D]
    bp0 = bp + hc0.astype(np.float64) @ Ut.astype(np.float64)
    initrow = np.concatenate([np.asarray(h_init), np.asarray(c_init)], axis=1)
    return dict(
        Wp=Wp.astype(BF16), Ut=Ut.astype(BF16),
        bias=np.ascontiguousarray(bp, np.float32),
        bias0=np.ascontiguousarray(bp0.astype(np.float32)),
        cinit=np.ascontiguousarray(np.asarray(c_init), np.float32),
        initrow=np.ascontiguousarray(initrow, np.float32),
    )


def _prep_core(inputs, r, levels, ns):
    """Per-core x transpose/cast and gather-index remap."""
    T = ns // P
    x = np.asarray(inputs["tensor"])[:, r * ns:(r + 1) * ns, :]   # [L, ns, DIN]
    xT = np.ascontiguousarray(x.transpose(0, 2, 1)).astype(BF16)  # [L, DIN, ns]
    idx = np.asarray(inputs["indices"])[:, r * ns:(r + 1) * ns, :].astype(np.int64)
    rem = np.where(idx < 0, 0, (idx // ns) * (ns + 1) + 1 + (idx % ns))
    # [L, ns, 2] -> [L, T, P, 2] -> [P, L, T, 2] -> [P, L*2T]
    g = rem.reshape(levels, T, P, 2).transpose(2, 0, 1, 3).reshape(P, levels * 2 * T)
    return dict(xT=xT, gidx=np.ascontiguousarray(g, np.int32))


def _run(inputs, trace=False, levels=L, n_total=N, n_cores=NCORES):
    from concourse import bass_utils

    ns = n_total // n_cores
    key = (levels, ns, n_cores)
    if key not in _CACHE:
        _CACHE[key] = _build(levels, ns, n_cores)
    nc = _CACHE[key]

    shared = _prep_shared(inputs)
    in_maps = []
    for r in range(n_cores):
        m = dict(shared)
        m.update(_prep_core(inputs, r, levels, ns))
        in_maps.append(m)

    res = bass_utils.run_bass_kernel_spmd(
        nc, in_maps, core_ids=list(range(n_cores)), trace=trace)
    res_h = np.concatenate([res.results[r]["resh"] for r in range(n_cores)], axis=1)
    res_c = np.concatenate([res.results[r]["resc"] for r in range(n_cores)], axis=1)
    return res_h, res_c, res


def kernel(**inputs):
    res_h, res_c, _ = _run(inputs)
    return res_h, res_c


# revision 4
# speedup vs baseline: 1.1762x; 1.1762x over previous
"""Binary TreeLSTM on 8 trn2 cores — v2.

vs v1: child-state exchange rows are [h:bf16(256B) | c:fp32(512B)] (768B,
25% smaller AllGather), and the per-level child gather uses the gpsimd
dma_gather fast path: h is gathered TRANSPOSED straight into the matmul
lhsT layout (no PE transposes / PSUM evacuations), c is gathered
node-major. Two SWDGE queues run the two gathers in parallel.
"""

import numpy as np
import ml_dtypes

L, N, DIN, DOUT = 24, 8192, 256, 128
NCORES = 8
NS = N // NCORES
P = 128
BF16 = ml_dtypes.bfloat16

_CACHE = {}


def _build(levels, ns, n_cores):
    import concourse.bass as bass
    import concourse.bacc as bacc
    import concourse.tile as tile
    import concourse.mybir as mybir
    from concourse.masks import make_identity  # noqa: F401

    f32 = mybir.dt.float32
    bf16 = mybir.dt.bfloat16
    i16 = mybir.dt.int16
    SIG = mybir.ActivationFunctionType.Sigmoid
    TANH = mybir.ActivationFunctionType.Tanh

    T = ns // P                      # node tiles per core
    NI = 2 * T * P                   # gathered rows per level (2048)
    V = n_cores * (ns + 1)           # table rows
    ROWB = 384                       # bf16 elems per row: h 128 | c 256 (fp32 pun)
    G = 640

    nc = bacc.Bacc("TRN2", target_bir_lowering=False, debug=False,
                   num_devices=n_cores, num_swdge_queues=2)

    xT_in = nc.dram_tensor("xT", [levels, DIN, ns], bf16, kind="ExternalInput")
    gidx_in = nc.dram_tensor("gidx16", [P, levels * (NI // 16)], i16,
                             kind="ExternalInput")
    Wp_in = nc.dram_tensor("Wp", [DIN, G], bf16, kind="ExternalInput")
    Ut_in = nc.dram_tensor("Ut", [2 * DOUT, G], bf16, kind="ExternalInput")
    bias_in = nc.dram_tensor("bias", [1, G], f32, kind="ExternalInput")
    bias0_in = nc.dram_tensor("bias0", [1, G], f32, kind="ExternalInput")
    cinit_in = nc.dram_tensor("cinit", [1, DOUT], f32, kind="ExternalInput")
    initrow_in = nc.dram_tensor("initrow", [1, ROWB], bf16, kind="ExternalInput")
    resh_out = nc.dram_tensor("resh", [levels, ns, DOUT], f32, kind="ExternalOutput")
    resc_out = nc.dram_tensor("resc", [levels, ns, DOUT], f32, kind="ExternalOutput")

    with tile.TileContext(nc) as tc:
        with (
            tc.tile_pool(name="const", bufs=1) as cp,
            tc.tile_pool(name="xp", bufs=3) as xp,
            tc.tile_pool(name="gp", bufs=2) as gp,
            tc.tile_pool(name="sp", bufs=2) as sp,
            tc.tile_pool(name="psum", bufs=3, space="PSUM") as psp,
            tc.tile_pool(name="dram", bufs=2, space="DRAM") as dp,
        ):
            # --- constants ---
            Wp0 = cp.tile([P, G], bf16)
            Wp1 = cp.tile([P, G], bf16)
            Ut0 = cp.tile([P, G], bf16)
            Ut1 = cp.tile([P, G], bf16)
            nc.sync.dma_start(out=Wp0[:], in_=Wp_in[0:P, :])
            nc.sync.dma_start(out=Wp1[:], in_=Wp_in[P:2 * P, :])
            nc.sync.dma_start(out=Ut0[:], in_=Ut_in[0:P, :])
            nc.sync.dma_start(out=Ut1[:], in_=Ut_in[P:2 * P, :])
            bias_t = cp.tile([1, G], f32)
            bias0_t = cp.tile([1, G], f32)
            cinit_t = cp.tile([1, DOUT], f32)
            nc.sync.dma_start(out=bias_t[:], in_=bias_in[:])
            nc.sync.dma_start(out=bias0_t[:], in_=bias0_in[:])
            nc.sync.dma_start(out=cinit_t[:], in_=cinit_in[:])
            gidx_t = cp.tile([P, levels * (NI // 16)], i16)
            nc.sync.dma_start(out=gidx_t[:], in_=gidx_in[:])
            ones1 = cp.tile([1, P], f32)
            nc.vector.memset(ones1[:], 1.0)
            cin_ps = psp.tile([P, DOUT], f32, space="PSUM")
            nc.tensor.matmul(out=cin_ps[:], lhsT=ones1[:], rhs=cinit_t[:],
                             start=True, stop=True)
            cinit_b = cp.tile([P, DOUT], f32)
            nc.vector.tensor_copy(out=cinit_b[:], in_=cin_ps[:])

            cc_in = dp.tile([ns + 1, ROWB], bf16, bufs=1)
            nc.sync.dma_start(out=cc_in[0:1, :], in_=initrow_in[:])

            prev_tbl = None
            for l in range(levels):
                xk0 = xp.tile([P, ns], bf16)
                xk1 = xp.tile([P, ns], bf16)
                nc.sync.dma_start(out=xk0[:], in_=xT_in[l, 0:P, :])
                nc.sync.dma_start(out=xk1[:], in_=xT_in[l, P:2 * P, :])

                if l > 0:
                    idxs = gidx_t[:, l * (NI // 16):(l + 1) * (NI // 16)]
                    hT = gp.tile([P, NI], bf16)
                    nc.gpsimd.dma_gather(
                        out_ap=hT[:].unsqueeze(1),           # [128, 1, NI]
                        in_ap=prev_tbl[:, 0:DOUT],           # [V, 128] step ROWB
                        idxs_ap=idxs, num_idxs=NI, num_idxs_reg=NI,
                        elem_size=DOUT, elem_step=ROWB,
                        transpose=True, queue_num=0,
                    )
                    cg = gp.tile([P, NI], f32)
                    cg3 = cg.rearrange("p (s d) -> p s d", d=P)
                    ctbl = prev_tbl[:].bitcast(f32)          # [V, 192]
                    nc.gpsimd.dma_gather(
                        out_ap=cg3[:],                       # [128, 16, 128]
                        in_ap=ctbl[:, DOUT // 2:ROWB // 2],  # [V, 128] f32
                        idxs_ap=idxs, num_idxs=NI, num_idxs_reg=NI,
                        elem_size=DOUT, elem_step=ROWB // 2,
                        transpose=False, queue_num=1,
                    )
                    cg4 = cg.rearrange("p (t c d) -> p t c d", c=2, d=P)

                sig_s = sp.tile([P, T * 512], f32)
                u_s = sp.tile([P, T * 128], f32)
                bt = bias0_t if l == 0 else bias_t

                for t in range(T):
                    ts = slice(t * P, (t + 1) * P)
                    pg0 = psp.tile([P, 512], f32, space="PSUM")
                    pg1 = psp.tile([P, 128], f32, space="PSUM")
                    last0 = l == 0
                    nc.tensor.matmul(out=pg0[:], lhsT=ones1[:], rhs=bt[:, 0:512],
                                     start=True, stop=False)
                    nc.tensor.matmul(out=pg0[:], lhsT=xk0[:, ts], rhs=Wp0[:, 0:512],
                                     start=False, stop=False)
                    nc.tensor.matmul(out=pg0[:], lhsT=xk1[:, ts], rhs=Wp1[:, 0:512],
                                     start=False, stop=last0)
                    nc.tensor.matmul(out=pg1[:], lhsT=ones1[:], rhs=bt[:, 512:640],
                                     start=True, stop=False)
                    nc.tensor.matmul(out=pg1[:], lhsT=xk0[:, ts], rhs=Wp0[:, 512:640],
                                     start=False, stop=False)
                    nc.tensor.matmul(out=pg1[:], lhsT=xk1[:, ts], rhs=Wp1[:, 512:640],
                                     start=False, stop=last0)
                    if l > 0:
                        h0T = hT[:, (2 * t) * P:(2 * t + 1) * P]
                        h1T = hT[:, (2 * t + 1) * P:(2 * t + 2) * P]
                        nc.tensor.matmul(out=pg0[:], lhsT=h0T,
                                         rhs=Ut0[:, 0:512], start=False, stop=False)
                        nc.tensor.matmul(out=pg0[:], lhsT=h1T,
                                         rhs=Ut1[:, 0:512], start=False, stop=True)
                        nc.tensor.matmul(out=pg1[:], lhsT=h0T,
                                         rhs=Ut0[:, 512:640], start=False, stop=False)
                        nc.tensor.matmul(out=pg1[:], lhsT=h1T,
                                         rhs=Ut1[:, 512:640], start=False, stop=True)

                    nc.scalar.activation(out=sig_s[:, t * 512:(t + 1) * 512],
                                         in_=pg0[:], func=SIG)
                    nc.scalar.activation(out=u_s[:, t * 128:(t + 1) * 128],
                                         in_=pg1[:], func=TANH)

                # --- batched elementwise ---
                sig4 = sig_s.rearrange("p (t g d) -> p t g d", t=T, g=4, d=128)
                f1v, f2v = sig4[:, :, 0, :], sig4[:, :, 1, :]
                iv, ov = sig4[:, :, 2, :], sig4[:, :, 3, :]
                uv = u_s.rearrange("p (t d) -> p t d", d=128)[:]
                if l > 0:
                    c0v = cg4[:, :, 0, :]
                    c1v = cg4[:, :, 1, :]
                else:
                    c0v = cinit_b[:].unsqueeze(1).to_broadcast([P, T, 128])
                    c1v = c0v

                h_s = sp.tile([P, T * 128], f32)
                c_s = sp.tile([P, T * 128], f32)
                hb_s = sp.tile([P, T * 128], bf16)
                tiu = sp.tile([P, T * 128], f32)
                t2 = sp.tile([P, T * 128], f32)
                t3 = sp.tile([P, T * 128], f32)
                tnh = sp.tile([P, T * 128], f32)
                tiu3 = tiu.rearrange("p (t d) -> p t d", d=128)
                t23 = t2.rearrange("p (t d) -> p t d", d=128)
                t33 = t3.rearrange("p (t d) -> p t d", d=128)
                nc.vector.tensor_mul(out=tiu3[:], in0=iv, in1=uv)
                nc.vector.tensor_mul(out=t23[:], in0=f1v, in1=c0v)
                nc.vector.tensor_mul(out=t33[:], in0=f2v, in1=c1v)
                nc.vector.tensor_add(out=t2[:], in0=t2[:], in1=t3[:])
                nc.vector.tensor_add(out=c_s[:], in0=tiu[:], in1=t2[:])
                nc.scalar.activation(out=tnh[:], in_=c_s[:], func=TANH)
                nc.vector.tensor_mul(out=h_s[:].rearrange("p (t d) -> p t d", d=128),
                                     in0=ov, in1=tnh.rearrange("p (t d) -> p t d", d=128)[:])
                nc.vector.tensor_copy(out=hb_s[:], in_=h_s[:])

                # --- outputs + exchange ---
                nc.sync.dma_start(
                    out=resh_out[l].rearrange("(t p) d -> p t d", p=P),
                    in_=h_s.rearrange("p (t d) -> p t d", d=128)[:])
                nc.sync.dma_start(
                    out=resc_out[l].rearrange("(t p) d -> p t d", p=P),
                    in_=c_s.rearrange("p (t d) -> p t d", d=128)[:])
                if l < levels - 1:
                    nc.sync.dma_start(
                        out=cc_in[1:1 + ns, 0:DOUT].rearrange("(t p) d -> p t d", p=P),
                        in_=hb_s.rearrange("p (t d) -> p t d", d=128)[:])
                    ccf = cc_in[:].bitcast(f32)              # [ns+1, 192]
                    nc.sync.dma_start(
                        out=ccf[1:1 + ns, DOUT // 2:ROWB // 2]
                            .rearrange("(t p) d -> p t d", p=P),
                        in_=c_s.rearrange("p (t d) -> p t d", d=128)[:])
                    tbl = dp.tile([V, ROWB], bf16,
                                  addr_space="Shared" if n_cores > 4 else "Local")
                    nc.gpsimd.collective_compute(
                        "AllGather", mybir.AluOpType.bypass,
                        replica_groups=[list(range(n_cores))],
                        ins=[cc_in[:].opt()], outs=[tbl[:].opt()],
                    )
                    prev_tbl = tbl

    nc.compile()
    return nc


def _prep_shared(inputs):
    W_w, W_b = inputs["W_w"], inputs["W_b"]
    U_f1, U_f2, U_iuo = inputs["U_f1"], inputs["U_f2"], inputs["U_iuo"]
    h_init, c_init = inputs["h_init"], inputs["c_init"]
    D = DOUT
    Wt = np.asarray(W_w).T
    Wf, Wi, Wu, Wo = Wt[:, 0:D], Wt[:, D:2 * D], Wt[:, 2 * D:3 * D], Wt[:, 3 * D:4 * D]
    Wp = np.concatenate([Wf, Wf, Wi, Wo, Wu], axis=1)
    b = np.asarray(W_b)
    bp = np.concatenate([b[0:D], b[0:D], b[D:2 * D], b[3 * D:4 * D],
                         b[2 * D:3 * D]])[None, :]
    Ut = np.concatenate([np.asarray(U_f1).T, np.asarray(U_f2).T,
                         np.asarray(U_iuo).T[:, 0:D],
                         np.asarray(U_iuo).T[:, 2 * D:3 * D],
                         np.asarray(U_iuo).T[:, D:2 * D]], axis=1)
    hc0 = np.concatenate([np.asarray(h_init), np.asarray(h_init)], axis=1)
    bp0 = bp + hc0.astype(np.float64) @ Ut.astype(np.float64)
    # init row: h as bf16 (256B) | c as fp32 punned to bf16 pairs (512B)
    hb = np.asarray(h_init).astype(BF16).reshape(-1).view(np.uint16)
    cb = np.ascontiguousarray(np.asarray(c_init), np.float32).reshape(-1).view(np.uint16)
    initrow = np.concatenate([hb, cb])[None, :].view(BF16)
    return dict(
        Wp=Wp.astype(BF16), Ut=Ut.astype(BF16),
        bias=np.ascontiguousarray(bp, np.float32),
        bias0=np.ascontiguousarray(bp0.astype(np.float32)),
        cinit=np.ascontiguousarray(np.asarray(c_init), np.float32),
        initrow=np.ascontiguousarray(initrow),
    )


def _prep_core(inputs, r, levels, ns):
    T = ns // P
    NI = 2 * T * P
    x = np.asarray(inputs["tensor"])[:, r * ns:(r + 1) * ns, :]
    xT = np.ascontiguousarray(x.transpose(0, 2, 1)).astype(BF16)
    idx = np.asarray(inputs["indices"])[:, r * ns:(r + 1) * ns, :].astype(np.int64)
    rem = np.where(idx < 0, 0, (idx // ns) * (ns + 1) + 1 + (idx % ns))
    # gather order i = (t*2+ch)*128 + p ; wrapped int16 [q=i%16, s=i//16]
    arr = rem.reshape(levels, T, P, 2).transpose(0, 1, 3, 2).reshape(levels, NI)
    blk = arr.reshape(levels, NI // 16, 16).transpose(0, 2, 1)   # [L, 16, NI/16]
    g16 = np.tile(blk, (1, P // 16, 1)).transpose(1, 0, 2).reshape(P, levels * (NI // 16))
    return dict(xT=xT, gidx16=np.ascontiguousarray(g16, np.int16))


def _run(inputs, trace=False, levels=L, n_total=N, n_cores=NCORES):
    from concourse import bass_utils

    ns = n_total // n_cores
    key = (levels, ns, n_cores)
    if key not in _CACHE:
        _CACHE[key] = _build(levels, ns, n_cores)
    nc = _CACHE[key]

    shared = _prep_shared(inputs)
    in_maps = []
    for r in range(n_cores):
        m = dict(shared)
        m.update(_prep_core(inputs, r, levels, ns))
        in_maps.append(m)

    res = bass_utils.run_bass_kernel_spmd(
        nc, in_maps, core_ids=list(range(n_cores)), trace=trace)
    res_h = np.concatenate([res.results[r]["resh"] for r in range(n_cores)], axis=1)
    res_c = np.concatenate([res.results[r]["resc"] for r in range(n_cores)], axis=1)
    return res_h, res_c, res


def kernel(**inputs):
    res_h, res_c, _ = _run(inputs)
    return res_h, res_c
